# revision 50
# baseline (speedup 1.0000x reference)
"""GAT message-passing kernel for 8 Trainium2 NeuronCores.

Strategy (dst-sharded padded-CSR, no collectives):
  - Host: shard nodes into 8 contiguous ranges balanced by in-edge count.
    Edges follow their dst node; each core computes output rows for its own
    nodes only, so no cross-core reduction is needed.
  - Device, per core:
      Phase 1: project features -> per-node fp16 table rows
               [feat(64) | el(4) | pad] (256B rows) written to a DRAM table;
               el folded into the projection matmul via an augmented weight
               matrix.  Table rows are laid out partition-major per padded
               core block (row = base_c + p*T + t) so each slab write is one
               contiguous run per SBUF partition.  A second pass computes er
               for the core's own nodes.
      Phase 2: for each tile of 128 dst nodes, dma_gather the fp16 table rows
               of their (padded) src neighbor lists, compute
               ee = exp(leaky_relu(el_src + er_dst)) on DVE/ACT, multiply into
               the gathered feats, and segment-reduce along the free dim.
               Softmax normalization is deferred: out = (sum ee*feat)/(sum ee).
  - int16 gather indices can't address all rows, so the table is split at a
    padded-core-block boundary (A = first kstar blocks <= 32767 rows, B =
    rest) and each node's neighbor list is split into A/B sublists; nodes are
    tiled grouped by A-degree so per-tile padding is small.  Dummy rows with
    el=-60000 make padded slots contribute exactly zero (ee=0).
  - Per-core trailing-pad trim: k-slots beyond this core's tile max degree
    are filled with idx=-1 and skipped by the gather ucode; the per-call
    valid count is loaded into a Pool GPR (num_idxs_reg).  A DVE memset
    pre-poisons the el region so skipped columns contribute ee=0.
"""

import os
import sys
from contextlib import ExitStack

import numpy as np

sys.path.insert(0, "/opt/trn_rl_repo")

# ---------------- problem constants (hardcoded per spec) ----------------
N_NODES = 50000
N_EDGES = 1600000
IN_DIM = 128
HEADS = 4
HID = 16
FEAT = HEADS * HID  # 64
NEG_SLOPE = 0.2
NCORES = 8
P = 128
ROW = 128          # table row size in fp16 elements (256 B)
EL_OFF = 64        # el lives at row[64:68]
NEG_BIG = -60000.0  # finite in fp16; exp(leaky(x)) == 0 well before here

# tunables
CMAX = int(os.environ.get("GAT_CMAX", "32"))  # max gather columns (k-slots) per unit
THRESH_CAP = 32767  # max int16 index
NQUEUES = int(os.environ.get("GAT_NQUEUES", "4"))
DMA_SCRATCH = int(os.environ.get("GAT_SCRATCH", "16384"))
CALLMAX = int(os.environ.get("GAT_CALLMAX", "4"))
CHAIN = os.environ.get("GAT_CHAIN", "1") == "1"
# outstanding gather calls per SWDGE queue; ring holds ~256 descs/lane/queue
# and a call emits 16*ck+1, so DEPTH*(16*CALLMAX+1) must stay under ~256
CHAIND = int(os.environ.get("GAT_CHAIND", "3"))
GBUFS = int(os.environ.get("GAT_GBUFS", "10"))
TRIM = os.environ.get("GAT_TRIM", "1") == "1"
NDUM_W = int(os.environ.get("GAT_NDUM", "128"))
SKIP_P2 = os.environ.get("GAT_SKIP_P2", "0") == "1"
SKIP_GATHER = os.environ.get("GAT_SKIP_GATHER", "0") == "1"
SKIP_COMPUTE = os.environ.get("GAT_SKIP_COMPUTE", "0") == "1"


def _round_up(x, m):
    return (x + m - 1) // m * m


# ---------------- host-side planning ----------------

class Plan:
    pass


def make_plan(src, dst, n_nodes, n_edges, ncores, thresh_cap=THRESH_CAP,
              cmax=CMAX):
    """Pure-index planning: core shards, node order, tile schedule, gather
    index arrays."""
    src = np.asarray(src).astype(np.int64)
    dst = np.asarray(dst).astype(np.int64)
    N = n_nodes
    E = n_edges

    deg = np.bincount(dst, minlength=N)
    cum = np.cumsum(deg)
    # node-id boundaries for the 8 cores, balanced by edge count
    targets = (np.arange(1, ncores) * E) // ncores
    inner = np.searchsorted(cum, targets, side="left") + 1
    bnds = np.concatenate([[0], inner, [N]]).astype(np.int64)
    Lc = np.diff(bnds)
    assert (Lc > 0).all()

    # uniform padded core blocks of T tiles each
    T = int(max((Lc + P - 1) // P))
    L = T * P  # padded slots per core

    # A/B split at a padded-core-block boundary, prefix <= thresh_cap
    NDUM = NDUM_W  # dummy rows per side, spread so padding reads hit many banks
    kstar = int(thresh_cap + 1 - NDUM) // L  # blocks + dummies below 32768
    assert kstar >= 1, "first padded block alone exceeds int16 range"
    kstar = min(kstar, ncores)
    theta = int(bnds[kstar])          # node-id threshold for A membership
    Bstar = kstar * L                 # dummyA rows = [Bstar, Bstar+NDUM)
    NB = (ncores - kstar) * L         # B-side real-slot rows
    TROWS = _round_up(Bstar + NDUM + NB + NDUM, P)

    isA = src < theta
    a_deg = np.bincount(dst[isA], minlength=N)
    b_deg = deg - a_deg

    # per-core node ordering: by (-a_deg, -b_deg)
    order = np.full((ncores, L), -1, dtype=np.int64)
    slot = np.empty(N, dtype=np.int64)   # global slot (core-padded, tile-major)
    for c in range(ncores):
        ids = np.arange(bnds[c], bnds[c + 1])
        # banded 2D packing: coarse a-band first, then b desc — trims the
        # per-tile max over BOTH a_deg and b_deg
        o = ids[np.lexsort((-b_deg[ids], -(a_deg[ids] // 4)))]
        order[c, : len(o)] = o
        slot[o] = c * L + np.arange(len(o))

    # table row of node: partition-major within its padded core block
    sc = slot // L                        # core of each node
    j = slot % L                          # local slot
    base = sc * L + (sc >= kstar) * NDUM  # dummyA block before B blocks
    tblpos = base + (j % P) * T + (j // P)
    dummyA = Bstar                        # base of the A dummy block
    dummyB_row = Bstar + NDUM + NB        # base of the B dummy block

    # edges sorted by (dst, then A-first)
    eorder = np.lexsort((~isA, dst))
    s_sorted = src[eorder]
    tid_sorted = tblpos[s_sorted]            # table row of each edge's src
    seg_start = np.concatenate([[0], cum])   # per-dst segment starts

    # per-(core,tile) max a/b degree -> global schedule
    KA = np.zeros((ncores, T), dtype=np.int64)
    KB = np.zeros((ncores, T), dtype=np.int64)
    for c in range(ncores):
        for t in range(T):
            slots = order[c, t * P : (t + 1) * P]
            real = slots[slots >= 0]
            if len(real):
                KA[c, t] = a_deg[real].max()
                KB[c, t] = b_deg[real].max()
    KAg = KA.max(axis=0)
    KBg = KB.max(axis=0)

    # units: split each tile's (KAg, KBg) columns into chunks <= cmax
    units = []
    for t in range(T):
        parts = []
        if KAg[t] > 0:
            parts.append(("A", int(KAg[t])))
        if KBg[t] > 0:
            parts.append(("B", int(KBg[t])))
        tile_units = []
        cur = {"t": t, "calls": [], "cols": 0}
        for side, k in parts:
            k0 = 0
            while k0 < k:
                room = cmax - cur["cols"]
                if room == 0:
                    tile_units.append(cur)
                    cur = {"t": t, "calls": [], "cols": 0}
                    room = cmax
                ck = min(room, k - k0, CALLMAX)
                cur["calls"].append((side, cur["cols"], k0, ck))
                cur["cols"] += ck
                k0 += ck
        tile_units.append(cur)  # possibly empty (degenerate tile)
        for i, u in enumerate(tile_units):
            u["first"] = i == 0
            u["last"] = i == len(tile_units) - 1
        units.extend(tile_units)

    # per-core int16 index arrays, packed per unit/call in 16-partition wrap
    totcols = 8 * sum(u["cols"] for u in units)
    totcols = max(totcols, 8)
    idx_arr = np.zeros((ncores, 16, totcols), dtype=np.int16)
    call_off = []
    off = 0
    for u in units:
        offs = []
        for side, gcol, k0, ck in u["calls"]:
            offs.append(off)
            off += 8 * ck
        call_off.append(offs)

    ncalls = sum(len(u["calls"]) for u in units)
    counts = np.zeros((ncores, ncalls), dtype=np.int32)
    for c in range(ncores):
        colA_cache = {}
        colB_cache = {}
        for t in range(T):
            slots = order[c, t * P : (t + 1) * P]
            ok = slots >= 0
            sl = np.where(ok, slots, 0)
            al = np.where(ok, a_deg[sl], 0).astype(np.int64)
            bl = np.where(ok, b_deg[sl], 0).astype(np.int64)
            st = seg_start[sl]
            ka, kb = int(KAg[t]), int(KBg[t])
            if ka > 0:
                spread = (np.arange(P)[:, None] + np.arange(ka)[None, :]) % NDUM
                arr = dummyA + spread.astype(np.int64)
                tot = int(al.sum())
                if tot:
                    cs = np.concatenate([[0], np.cumsum(al)[:-1]])
                    flat = np.repeat(st, al) + (np.arange(tot) - np.repeat(cs, al))
                    mask = np.arange(ka)[None, :] < al[:, None]
                    arr[mask] = tid_sorted[flat]
                colA_cache[t] = arr.T  # [ka, 128]
            if kb > 0:
                spread = (np.arange(P)[:, None] + np.arange(kb)[None, :]) % NDUM
                arr = NB + spread.astype(np.int64)  # dummyB idxs = NB..NB+127
                tot = int(bl.sum())
                if tot:
                    cs = np.concatenate([[0], np.cumsum(bl)[:-1]])
                    flat = (np.repeat(st + al, bl)
                            + (np.arange(tot) - np.repeat(cs, bl)))
                    mask = np.arange(kb)[None, :] < bl[:, None]
                    arr[mask] = tid_sorted[flat] - (Bstar + NDUM)
                colB_cache[t] = arr.T
        ci = 0
        for ui, u in enumerate(units):
            t = u["t"]
            slots = order[c, t * P : (t + 1) * P]
            ok = slots >= 0
            sl = np.where(ok, slots, 0)
            kac = int(np.where(ok, a_deg[sl], 0).max()) if ok.any() else 0
            kbc = int(np.where(ok, b_deg[sl], 0).max()) if ok.any() else 0
            for (side, gcol, k0, ck), o in zip(u["calls"], call_off[ui]):
                cols = (colA_cache if side == "A" else colB_cache)[t]
                blk = cols[k0 : k0 + ck]  # [ck, 128] k-major
                kcore = kac if side == "A" else kbc
                valid = ck
                if TRIM:
                    valid = max(1, min(ck, kcore - k0))
                    blk = blk.copy()
                    blk[valid:, :] = -1
                counts[c, ci] = valid * P
                blk = blk.reshape(-1)
                assert blk.max() <= 32767 and blk.min() >= -1
                idx_arr[c, :, o : o + 8 * ck] = (
                    blk.reshape(-1, 16).T.astype(np.int16))
                ci += 1

    idx_full = np.tile(idx_arr, (1, 8, 1))  # [ncores, 128, totcols]

    p = Plan()
    p.N, p.E, p.ncores = N, E, ncores
    p.T, p.L, p.TROWS = T, L, TROWS
    p.kstar, p.Bstar, p.NB, p.NDUM = kstar, Bstar, NB, NDUM
    p.dummyA, p.dummyB_row = dummyA, dummyB_row
    p.order, p.tblpos = order, tblpos
    p.units, p.call_off, p.totcols = units, call_off, totcols
    p.idx_full = idx_full
    p.KAg, p.KBg = KAg, KBg
    p.counts, p.ncalls = counts, ncalls
    return p


# ---------------- device kernel builder ----------------

def build_nc(plan, num_cores, cmax=CMAX, reps=1):
    import concourse.bacc as bacc
    import concourse.bass as bass
    import concourse.tile as tile
    from concourse import mybir
    from concourse.tile import add_dep_helper

    f32 = mybir.dt.float32
    f16 = mybir.dt.float16
    i16 = mybir.dt.int16
    i32 = mybir.dt.int32
    Alu = mybir.AluOpType
    Act = mybir.ActivationFunctionType

    T, L, TROWS = plan.T, plan.L, plan.TROWS
    kstar, Bstar, NB = plan.kstar, plan.Bstar, plan.NB
    totcols = plan.totcols
    SLOTS = plan.ncores * L

    nc = bacc.Bacc("TRN2", target_bir_lowering=False, debug=False,
                   enable_asserts=False, num_devices=num_cores,
                   num_swdge_queues=NQUEUES,
                   dynamic_dma_scratch_size=DMA_SCRATCH)

    fT = nc.dram_tensor("fT", [P, SLOTS], f16, kind="ExternalInput").ap()
    fLT = nc.dram_tensor("fLT", [P, L], f16, kind="ExternalInput").ap()
    Waug = nc.dram_tensor("Waug", [P, 68], f16, kind="ExternalInput").ap()
    Wr = nc.dram_tensor("Wr", [P, 4], f16, kind="ExternalInput").ap()
    idx = nc.dram_tensor("idx", [P, totcols], i16, kind="ExternalInput").ap()
    biasm = nc.dram_tensor("biasm", [1, 16], f32, kind="ExternalInput").ap()
    cnt = nc.dram_tensor("cnt", [1, plan.ncalls], i32,
                         kind="ExternalInput").ap()
    out = nc.dram_tensor("out", [L, 16], f32, kind="ExternalOutput").ap()

    with tile.TileContext(nc) as tc, ExitStack() as ctx:
        if reps > 1:
            ctx.enter_context(tc.For_i(0, reps, 1))
        dpool = ctx.enter_context(tc.tile_pool(name="dram", bufs=1, space="DRAM"))
        wpool = ctx.enter_context(tc.tile_pool(name="wpool", bufs=1))
        # split A/B tables: A-side gathers only depend on A-side writes, so
        # they can start while the B blocks are still being projected
        NDUM = plan.NDUM
        tableA = dpool.tile([Bstar + NDUM, ROW], f16)
        tableB = dpool.tile([NB + NDUM, ROW], f16)

        # resident small tiles
        waug_sb = wpool.tile([P, 68], f16, tag="waug")
        wr_sb = wpool.tile([P, 4], f16, tag="wr")
        biasm_sb = wpool.tile([P, 16], f32, tag="biasm")
        er_sb = wpool.tile([P, T, 4], f32, tag="er")
        idx_sb = wpool.tile([P, totcols], i16, tag="idx")
        cnt_sb = wpool.tile([1, plan.ncalls], i32, tag="cnt")

        nc.sync.dma_start(waug_sb[:], Waug)
        nc.sync.dma_start(wr_sb[:], Wr)
        nc.sync.dma_start(biasm_sb[:1, :], biasm)
        nc.gpsimd.partition_broadcast(biasm_sb[:], biasm_sb[:1, :])
        nc.sync.dma_start(idx_sb[:], idx)
        nc.sync.dma_start(cnt_sb[:], cnt)
        maxcalls = max((len(u["calls"]) for u in plan.units), default=1)
        cnt_regs = ([nc.gpsimd.alloc_register(f"cnt_reg{i}")
                     for i in range(maxcalls)] if TRIM else None)

        # ---- phase 1: build table (feat | el), one slab per core block ----
        with tc.tile_pool(name="slab", bufs=2) as slabp, \
             tc.tile_pool(name="stage", bufs=2) as stagep, \
             tc.tile_pool(name="ps1", bufs=4, space="PSUM") as psp:
            # dummy rows first (NDUM per side, spread across DRAM pages):
            # el = -60000 so padded gather slots contribute ee=0
            dmy = wpool.tile([P, ROW], f16, tag="dmy")
            nc.vector.memset(dmy[:], 0.0)
            nc.vector.memset(dmy[:, EL_OFF : EL_OFF + 4], NEG_BIG)
            for tbl, drow in ((tableA, Bstar), (tableB, NB)):
                for doff in range(0, plan.NDUM, P):
                    n = min(P, plan.NDUM - doff)
                    nc.sync.dma_start(
                        tbl[drow + doff : drow + doff + n, :], dmy[:n, :])

            for s in range(plan.ncores):
                slab = slabp.tile([P, L], f16, tag="slab")
                nc.sync.dma_start(slab[:], fT[:, s * L : (s + 1) * L])
                stage = stagep.tile([P, T, ROW], f16, tag="stage")
                for jj in range(T):
                    ps = psp.tile([P, 68], f32, tag="ps")
                    nc.tensor.matmul(out=ps[:],
                                     lhsT=slab[:, jj * P : (jj + 1) * P],
                                     rhs=waug_sb[:], start=True, stop=True)
                    nc.scalar.activation(stage[:, jj, 0:68], ps[:], Act.Copy)
                if s < kstar:
                    dst_view = tableA[s * L : (s + 1) * L, :]
                else:
                    dst_view = tableB[(s - kstar) * L : (s - kstar + 1) * L, :]
                nc.sync.dma_start(
                    dst_view.rearrange("(p t) f -> p t f", p=P),
                    stage[:])

            # ---- phase 1b: er for local nodes (single fLT slab) ----
            fl = slabp.tile([P, L], f16, tag="fl")
            nc.sync.dma_start(fl[:], fLT)
            for t in range(T):
                pse = psp.tile([P, 4], f32, tag="pse")
                nc.tensor.matmul(out=pse[:], lhsT=fl[:, t * P : (t + 1) * P],
                                 rhs=wr_sb[:], start=True, stop=True)
                nc.vector.tensor_copy(er_sb[:, t, :], pse[:])

        # ---- phase 2: gather + attention + aggregate ----
        tabA = tableA[:, :]
        tabB = tableB[:, :]
        with tc.tile_pool(name="gp", bufs=GBUFS) as gp, \
             tc.tile_pool(name="eep", bufs=2) as eep, \
             tc.tile_pool(name="accp", bufs=2) as accp, \
             tc.tile_pool(name="tmpp", bufs=2) as tmpp, \
             tc.tile_pool(name="finp", bufs=2) as finp:
            qn = 0
            last_call = {}
            acc = accd = None
            ci = 0
            # Pre-create and pre-poison G tiles PRE units ahead of their
            # gathers: the el memset runs on DVE, and emitting it just
            # before the gather chains each unit's desc-gen behind the
            # previous unit's compute. Preponed, the memset clears the DVE
            # queue long before its gather issues.
            PRE = int(os.environ.get("GAT_PRE", "8"))
            gtiles = {}
            gcreated = [0]
            main_path = not (SKIP_P2 or SKIP_GATHER or SKIP_COMPUTE)

            def prep_G(ui2):
                u2 = plan.units[ui2]
                if u2["cols"] == 0:
                    return
                G2 = gp.tile([P, cmax, ROW], f16, tag="G")
                gtiles[ui2] = G2
                if TRIM:
                    if gcreated[0] < GBUFS:
                        # first use of each pool buffer: clear ALL of it —
                        # skipped columns must hold finite fp16, and fresh
                        # SBUF can alias NaN bit patterns
                        nc.vector.memset(G2[:], 0.0)
                    nc.vector.memset(
                        G2[:, :u2["cols"], EL_OFF : EL_OFF + 4], NEG_BIG)
                gcreated[0] += 1

            if main_path:
                for ui2 in range(min(PRE, len(plan.units))):
                    prep_G(ui2)

            def do_gather(G, u, ui):
                nonlocal qn, ci
                nu = len(u["calls"])
                if TRIM and nu:
                    # one batched load of this unit's per-call valid counts
                    nc.gpsimd.reg_load(cnt_regs[:nu],
                                       cnt_sb[0:1, ci : ci + nu])
                for cj, ((side, gcol, k0, ck), o) in enumerate(
                        zip(u["calls"], plan.call_off[ui])):
                    tab = tabA if side == "A" else tabB
                    nreg = ck * P
                    if TRIM:
                        nreg = cnt_regs[cj]
                    gi = nc.gpsimd.dma_gather(
                        G[:, gcol : gcol + ck, :], tab,
                        idx_sb[:, o : o + 8 * ck],
                        ck * P, nreg, ROW, queue_num=qn)
                    if CHAIN:
                        hist = last_call.setdefault(qn, [])
                        if len(hist) >= CHAIND:
                            add_dep_helper(gi.ins, hist[-CHAIND],
                                           reason="swdge ring throttle")
                        hist.append(gi.ins)
                        del hist[:-CHAIND]
                    qn = (qn + 1) % NQUEUES
                    ci += 1

            for ui, u in enumerate(plan.units):
                if SKIP_P2:
                    break
                t = u["t"]
                C = u["cols"]
                if SKIP_GATHER or SKIP_COMPUTE:
                    G = gp.tile([P, cmax, ROW], f16, tag="G")
                    if not SKIP_GATHER:
                        if TRIM:
                            nc.vector.memset(
                                G[:, :max(C, 1), EL_OFF : EL_OFF + 4],
                                NEG_BIG)
                        do_gather(G, u, ui)
                    if SKIP_COMPUTE:
                        continue
                    if u["last"]:
                        fin = finp.tile([P, 16], f32, tag="fin")
                        nc.vector.tensor_reduce(
                            fin[:], G[:, :max(C, 1), 0:16]
                            .rearrange("p c f -> p f c"),
                            mybir.AxisListType.XY, Alu.add)
                        nc.sync.dma_start(out[t * P : (t + 1) * P, :], fin[:])
                    continue
                if u["first"]:
                    acc = accp.tile([P, FEAT], f32, tag="acc")
                    accd = accp.tile([P, 4], f32, tag="accd")
                if ui + PRE < len(plan.units):
                    prep_G(ui + PRE)
                if C > 0:
                    G = gtiles[ui]
                    do_gather(G, u, ui)
                    ee = eep.tile([P, cmax, 4], f16, tag="ee")
                    # e = el_src + er_dst
                    nc.vector.tensor_tensor(
                        ee[:, :C, :], G[:, :C, EL_OFF : EL_OFF + 4],
                        er_sb[:, t : t + 1, :].to_broadcast([P, C, 4]),
                        Alu.add)
                    # leaky relu: max(x, 0.2x)
                    nc.vector.scalar_tensor_tensor(
                        ee[:, :C, :], ee[:, :C, :], NEG_SLOPE, ee[:, :C, :],
                        Alu.mult, Alu.max)
                    nc.scalar.activation(ee[:, :C, :], ee[:, :C, :], Act.Exp)
                    # msg = feat * ee (broadcast over hid dim), in-place fp16
                    nc.vector.tensor_tensor(
                        G[:, :C, 0:FEAT].rearrange("p c (h d) -> p c h d", h=HEADS),
                        G[:, :C, 0:FEAT].rearrange("p c (h d) -> p c h d", h=HEADS),
                        ee[:, :C, :].unsqueeze(-1).to_broadcast([P, C, HEADS, HID]),
                        Alu.mult)
                    # k-reduction (innermost = k via strided view)
                    msum = G[:, :C, 0:FEAT].rearrange("p c f -> p f c")
                    dsum = ee[:, :C, :].rearrange("p c h -> p h c")
                    if u["first"]:
                        nc.vector.tensor_reduce(acc[:], msum,
                                                mybir.AxisListType.X, Alu.add)
                        nc.vector.tensor_reduce(accd[:], dsum,
                                                mybir.AxisListType.X, Alu.add)
                    else:
                        tmp = tmpp.tile([P, FEAT], f32, tag="tmp")
                        tmpd = tmpp.tile([P, 4], f32, tag="tmpd")
                        nc.vector.tensor_reduce(tmp[:], msum,
                                                mybir.AxisListType.X, Alu.add)
                        nc.vector.tensor_reduce(tmpd[:], dsum,
                                                mybir.AxisListType.X, Alu.add)
                        nc.vector.tensor_add(acc[:], acc[:], tmp[:])
                        nc.vector.tensor_add(accd[:], accd[:], tmpd[:])
                elif u["first"]:
                    nc.vector.memset(acc[:], 0.0)
                    nc.vector.memset(accd[:], 1.0)
                if u["last"]:
                    nc.vector.tensor_scalar_max(accd[:], accd[:], 1e-30)
                    rec = tmpp.tile([P, 4], f32, tag="rec")
                    nc.vector.reciprocal(rec[:], accd[:])
                    nc.vector.tensor_tensor(
                        acc[:].rearrange("p (h d) -> p h d", h=HEADS),
                        acc[:].rearrange("p (h d) -> p h d", h=HEADS),
                        rec[:].unsqueeze(-1).to_broadcast([P, HEADS, HID]),
                        Alu.mult)
                    fin = finp.tile([P, 16], f32, tag="fin")
                    nc.vector.tensor_reduce(
                        fin[:], acc[:].rearrange("p (h d) -> p d h", h=HEADS),
                        mybir.AxisListType.X, Alu.add)
                    nc.vector.scalar_tensor_tensor(
                        fin[:], fin[:], 1.0 / HEADS, biasm_sb[:],
                        Alu.mult, Alu.add)
                    nc.sync.dma_start(out[t * P : (t + 1) * P, :], fin[:])

    nc.compile()
    return nc


# ---------------- host wrapper ----------------

_CACHE = {}


def _get_plan_and_nc(src, dst, n_nodes, n_edges, ncores,
                     thresh_cap=THRESH_CAP, cmax=CMAX):
    key = (int(src[0]), int(src[-1]), int(dst[0]), int(dst[-1]),
           len(src), n_nodes, ncores, thresh_cap, cmax)
    if key not in _CACHE:
        plan = make_plan(src, dst, n_nodes, n_edges, ncores,
                         thresh_cap=thresh_cap, cmax=cmax)
        nc = build_nc(plan, ncores, cmax=cmax)
        _CACHE[key] = (plan, nc)
    return _CACHE[key]


def make_inputs(plan, features, W, attn_l, attn_r, bias):
    """Build per-core input maps from full inputs + plan."""
    features = np.asarray(features, dtype=np.float32)
    W = np.asarray(W, dtype=np.float32)
    attn_l = np.asarray(attn_l, dtype=np.float32)
    attn_r = np.asarray(attn_r, dtype=np.float32)
    bias = np.asarray(bias, dtype=np.float32)

    # augmented weights
    W3 = W.reshape(IN_DIM, HEADS, HID)
    Wl = np.einsum("ihd,hd->ih", W3, attn_l).astype(np.float16)
    Wr_ = np.einsum("ihd,hd->ih", W3, attn_r).astype(np.float16)
    Waug = np.concatenate([W.astype(np.float16), Wl], axis=1)  # [128, 68]
    biasm = bias.reshape(HEADS, HID).mean(axis=0).reshape(1, 16)
    biasm = np.ascontiguousarray(biasm, dtype=np.float32)

    # fT: [128, SLOTS] columns = features of node at global slot (0 for pads)
    SLOTS = plan.ncores * plan.L
    fT = np.zeros((IN_DIM, SLOTS), dtype=np.float16)
    featT16 = features.T.astype(np.float16)
    for c in range(plan.ncores):
        o = plan.order[c]
        real = o >= 0
        fT[:, c * plan.L + np.nonzero(real)[0]] = featT16[:, o[real]]
    fT = np.ascontiguousarray(fT)

    in_maps = []
    for c in range(plan.ncores):
        fLT = np.zeros((IN_DIM, plan.L), dtype=np.float16)
        o = plan.order[c]
        real = o >= 0
        fLT[:, np.nonzero(real)[0]] = featT16[:, o[real]]
        in_maps.append({
            "fT": fT,
            "fLT": np.ascontiguousarray(fLT),
            "Waug": Waug,
            "Wr": Wr_,
            "idx": np.ascontiguousarray(plan.idx_full[c]),
            "biasm": biasm,
            "cnt": np.ascontiguousarray(plan.counts[c][None, :]),
        })
    return in_maps


def unshard_output(plan, outs):
    """outs: list of per-core {'out': [L,16]} -> full [N,16]."""
    res = np.empty((plan.N, 16), dtype=np.float32)
    for c in range(plan.ncores):
        o = plan.order[c]
        real = o >= 0
        res[o[real]] = outs[c]["out"][np.nonzero(real)[0]]
    return res


def kernel(features, W, attn_l, attn_r, bias, src, dst):
    from concourse.bass_utils import run_bass_kernel_spmd

    src = np.asarray(src)
    dst = np.asarray(dst)
    plan, nc = _get_plan_and_nc(src, dst, N_NODES, N_EDGES, NCORES)
    in_maps = make_inputs(plan, features, W, attn_l, attn_r, bias)
    res = run_bass_kernel_spmd(nc, in_maps, core_ids=list(range(NCORES)))
    return unshard_output(plan, res.results)
